# revision 1
# baseline (speedup 1.0000x reference)
"""Trainium2 Bass kernel for nn_Attention_87857851006980.

Sharding: 8 cores = 4 batches x 2 head-halves. Core c handles batch c//2,
heads [0..4) (even c) or [4..8) (odd c). Each core computes qkv for its
heads (full-d contraction), the conv/spe branches for its heads' channels,
attention for its heads, and a partial nn1 over its 512 channels; a
pair-wise ReduceScatter then sums the nn1 partials, leaving each core with
half of the output channels for its batch. Host gathers/transposes.

All heavy math runs on-device; the host only slices/transposes inputs
(layout) and folds BN/bias constants into weight tensors.
"""
import sys
sys.path.insert(0, "/opt/trn_rl_repo")
import numpy as np

import concourse.bacc as bacc
import concourse.bass as bass
import concourse.tile as tile
import concourse.mybir as mybir
from concourse.bass_utils import run_bass_kernel_spmd

F32 = mybir.dt.float32
F32R = mybir.dt.float32r
BF16 = mybir.dt.bfloat16
AF = mybir.ActivationFunctionType
ADD = mybir.AluOpType.add
MULT = mybir.AluOpType.mult

B, D, N, H, HD = 4, 1024, 2304, 8, 128
PS = 48          # image side; N = PS*PS
PP = PS + 2      # padded side
NPAD = PP * PP + 2  # 2502: +2 so the (dy,dx)=(1,1) view of the last row-chunk stays in bounds
HPC = 4          # heads per core
CH = HPC * HD    # 512 channels per core
EPS = 1e-5
SCALE = D ** -0.5

# n-chunks of the free dimension (all >=256 so f32r runs at full rate)
CHUNKS = [(0, 512), (512, 512), (1024, 512), (1536, 512), (2048, 256)]
NJ = N // 128    # 18 key blocks
EB = 8           # qkv output blocks per core: 4 q + 4 k (v folded into wvsum)

USE_COLLECTIVE = True


def _build(single=False):
    use_cc = USE_COLLECTIVE and not single
    nc = bacc.Bacc("TRN2", target_bir_lowering=False, debug=False,
                   num_devices=1 if single else 8)

    # ---- DRAM I/O ----
    x_dn = nc.dram_tensor("x_dn", [D, N], F32, kind="ExternalInput").ap()
    xpad_d = nc.dram_tensor("xpad", [CH, NPAD], F32, kind="ExternalInput").ap()
    wqkv_d = nc.dram_tensor("wqkv", [128, 8 * EB * 128], F32, kind="ExternalInput").ap()
    wvsum_d = nc.dram_tensor("wvsum", [128, 8 * HPC], F32, kind="ExternalInput").ap()
    bqk_d = nc.dram_tensor("bqk", [128, 8], F32, kind="ExternalInput").ap()
    vbias_d = nc.dram_tensor("vbias", [128, HPC], F32, kind="ExternalInput").ap()
    wconv_d = nc.dram_tensor("wconv", [128, HPC * 9 * 128], F32, kind="ExternalInput").ap()
    wspe_d = nc.dram_tensor("wspe", [128, HPC * 128], F32, kind="ExternalInput").ap()
    bn_s_d = nc.dram_tensor("bn_s", [128, HPC], F32, kind="ExternalInput").ap()
    bn_b_d = nc.dram_tensor("bn_b", [128, HPC], F32, kind="ExternalInput").ap()
    bnc_s_d = nc.dram_tensor("bnc_s", [128, HPC], F32, kind="ExternalInput").ap()
    bnc_b_d = nc.dram_tensor("bnc_b", [128, HPC], F32, kind="ExternalInput").ap()
    wnn1_d = nc.dram_tensor("wnn1", [128, HPC * D], F32, kind="ExternalInput").ap()
    bnn1_d = nc.dram_tensor("bnn1h", [128, 8], F32, kind="ExternalInput").ap()
    ones_d = nc.dram_tensor("onesc", [128, 2], F32, kind="ExternalInput").ap()
    ident_d = nc.dram_tensor("ident", [128, 128], F32, kind="ExternalInput").ap()
    if use_cc:
        out_d = nc.dram_tensor("out", [CH, N], F32, kind="ExternalOutput").ap()
    else:
        out_d = nc.dram_tensor("out", [D, N], F32, kind="ExternalOutput").ap()

    with tile.TileContext(nc) as tc:
      with tc.tile_pool(name="persist", bufs=1) as pp:
        # ---------- persistent tiles ----------
        qk_sb = pp.tile([128, 8 * N], F32R, tag="qk")   # q then k, 4 heads each
        vpT_sb = pp.tile([128, HPC * NJ * 128], BF16, tag="vpT")  # V' (cbr^T, then +v_spe)
        vcol_all = pp.tile([128, HPC * NJ], F32, tag="vcol_all")
        spe_row = pp.tile([1, HPC * 128], F32R, tag="spe_row")
        ones_sb = pp.tile([128, 2], F32R, tag="ones")
        ones_bf = pp.tile([128, 1], BF16, tag="ones_bf")
        ones_row = pp.tile([1, 128], F32R, tag="ones_row")
        ident_sb = pp.tile([128, 128], F32, tag="ident")
        bqk_sb = pp.tile([128, 8], F32, tag="bqk")
        vbias_sb = pp.tile([128, HPC], F32, tag="vbias")
        bn_s = pp.tile([128, HPC], F32, tag="bn_s")
        bn_b = pp.tile([128, HPC], F32, tag="bn_b")
        bnc_s = pp.tile([128, HPC], F32, tag="bnc_s")
        bnc_b = pp.tile([128, HPC], F32, tag="bnc_b")
        bnn1_sb = pp.tile([128, 8], F32, tag="bnn1")
        wspe_sb = pp.tile([128, HPC * 128], F32R, tag="wspe")
        wvsum_sb = pp.tile([128, 8 * HPC], F32R, tag="wvsum")
        ident_bf = pp.tile([128, 128], BF16, tag="ident_bf")

        # spe_bc tiles live from phase C1 until the C2 fold
        with tc.tile_pool(name="spb", bufs=4) as spb_pool:
          spe_bcs = []

          # ---- phase C1: conv + spe; vpT := cbr^T ----
          with tc.tile_pool(name="cvin", bufs=2) as cvin_pool, \
               tc.tile_pool(name="cvw", bufs=2) as cvw_pool, \
               tc.tile_pool(name="cbr", bufs=2) as cbr_pool, \
               tc.tile_pool(name="scr", bufs=1) as scr_pool, \
               tc.tile_pool(name="pcol", bufs=2) as pcol_pool, \
               tc.tile_pool(name="cps", bufs=2, space="PSUM") as cps, \
               tc.tile_pool(name="tps", bufs=3, space="PSUM") as tps, \
               tc.tile_pool(name="sps", bufs=1, space="PSUM") as sps:
            xp0 = cvin_pool.tile([128, NPAD], F32R, tag="xp")
            nc.sync.dma_start(xp0[:], xpad_d[0:128, :].bitcast(F32R))
            wcv0 = cvw_pool.tile([128, 9 * 128], F32R, tag="wcv")
            nc.sync.dma_start(wcv0[:], wconv_d[:, 0:9 * 128].bitcast(F32R))
            nc.sync.dma_start(ones_sb[:], ones_d[:].bitcast(F32R))
            nc.vector.tensor_copy(ones_bf[:], ones_sb[:, 0:1].bitcast(F32))
            nc.sync.dma_start(ones_row[:], ones_sb[:, 0:1])
            nc.sync.dma_start(ident_sb[:], ident_d[:])
            nc.sync.dma_start(bqk_sb[:], bqk_d[:])
            nc.sync.dma_start(vbias_sb[:], vbias_d[:])
            nc.sync.dma_start(bn_s[:], bn_s_d[:])
            nc.sync.dma_start(bn_b[:], bn_b_d[:])
            nc.sync.dma_start(bnc_s[:], bnc_s_d[:])
            nc.sync.dma_start(bnc_b[:], bnc_b_d[:])
            nc.sync.dma_start(bnn1_sb[:], bnn1_d[:])
            nc.sync.dma_start(wspe_sb[:], wspe_d[:].bitcast(F32R))
            nc.sync.dma_start(wvsum_sb[:], wvsum_d[:].bitcast(F32R))
            nc.vector.tensor_copy(ident_bf[:], ident_sb[:])

            for h in range(HPC):
                if h == 0:
                    xp, wcv = xp0, wcv0
                else:
                    xp = cvin_pool.tile([128, NPAD], F32R, tag="xp")
                    nc.sync.dma_start(xp[:],
                                      xpad_d[h * 128:(h + 1) * 128, :].bitcast(F32R))
                    wcv = cvw_pool.tile([128, 9 * 128], F32R, tag="wcv")
                    nc.sync.dma_start(
                        wcv[:], wconv_d[:, h * 9 * 128:(h + 1) * 9 * 128].bitcast(F32R))

                # spe branch: gelu(bn(x)) with running row-sum -> pooled -> spe row
                scr = scr_pool.tile([128, N], BF16, tag="scr")
                pcol = pcol_pool.tile([128, 1], F32, tag="pcol")
                interior = xp[:, PP + 1:PP + 1 + PS * PP].rearrange(
                    "p (r c) -> p r c", c=PP)[:, :, 0:PS].bitcast(F32)
                nc.scalar.activation(
                    scr[:].rearrange("p (r c) -> p r c", c=PS), interior,
                    AF.Gelu, bias=bn_b[:, h:h + 1], scale=bn_s[:, h:h + 1],
                    accum_out=pcol[:],
                )
                pcol_r = pcol_pool.tile([128, 1], F32R, tag="pcolr")
                nc.vector.tensor_copy(pcol_r[:], pcol[:])
                ps_spe = sps.tile([1, 128], F32, tag="spe")
                nc.tensor.matmul(ps_spe[:], pcol_r[:],
                                 wspe_sb[:, h * 128:(h + 1) * 128], start=True, stop=True)
                nc.vector.tensor_copy(spe_row[:, h * 128:(h + 1) * 128], ps_spe[:])
                # broadcast spe over partitions: spe_bc[p, c] = spe[c]
                ps_bc = sps.tile([128, 128], F32, tag="bc")
                nc.tensor.matmul(ps_bc[:], ones_row[:],
                                 spe_row[:, h * 128:(h + 1) * 128], start=True, stop=True)
                spe_bc = spb_pool.tile([128, 128], F32, tag="spb")
                nc.vector.tensor_copy(spe_bc[:], ps_bc[:])
                spe_bcs.append(spe_bc)

                # conv branch: 9 shifted matmuls, bn+gelu
                cbr = cbr_pool.tile([128, N], BF16, tag="cbr")
                for rc in range(6):  # 8 output rows per chunk
                    r0 = rc * 8
                    pc = cps.tile([128, 8 * PS], F32, tag="cv")
                    for oi, (dy, dx) in enumerate(
                            [(a, b) for a in (-1, 0, 1) for b in (-1, 0, 1)]):
                        base = (r0 + 1 + dy) * PP + 1 + dx
                        rhs = xp[:, base:base + 8 * PP].rearrange(
                            "p (r c) -> p r c", c=PP)[:, :, 0:PS]
                        nc.tensor.matmul(pc[:].rearrange("p (r c) -> p r c", c=PS),
                                         wcv[:, oi * 128:(oi + 1) * 128], rhs,
                                         start=(oi == 0), stop=(oi == 8))
                    nc.scalar.activation(cbr[:, r0 * PS:(r0 + 8) * PS], pc[:],
                                         AF.Gelu, bias=bnc_b[:, h:h + 1],
                                         scale=bnc_s[:, h:h + 1])
                for jb in range(NJ):
                    pt = tps.tile([128, 128], BF16, tag="tp")
                    nc.tensor.transpose(pt[:], cbr[:, jb * 128:(jb + 1) * 128], ident_bf[:])
                    nc.vector.tensor_copy(
                        vpT_sb[:, (h * NJ + jb) * 128:(h * NJ + jb + 1) * 128], pt[:])

          # ---- phase A: qkv projection (+ folded vsum rows) ----
          with tc.tile_pool(name="wqp", bufs=1) as wqp, \
               tc.tile_pool(name="vr", bufs=1) as vr_pool, \
               tc.tile_pool(name="xch", bufs=4) as xch_pool, \
               tc.tile_pool(name="qps", bufs=4, space="PSUM") as qps, \
               tc.tile_pool(name="vps", bufs=2, space="PSUM") as vps:
            wq_sb = wqp.tile([128, 8 * EB * 128], F32R)
            for dt in range(8):
                nc.sync.dma_start(
                    wq_sb[:, dt * EB * 128:(dt + 1) * EB * 128],
                    wqkv_d[:, dt * EB * 128:(dt + 1) * EB * 128].bitcast(F32R))
            vrow4 = vr_pool.tile([4, N], F32)
            for (n0, nw) in CHUNKS:
                xw = []
                for half in range(2):
                    t = xch_pool.tile([128, 4 * 512], F32R, tag="x")
                    src_ap = x_dn[half * 512:(half + 1) * 512, n0:n0 + nw].rearrange(
                        "(t p) n -> p t n", p=128)
                    nc.sync.dma_start(
                        t[:].rearrange("p (t n) -> p t n", t=4)[:, :, 0:nw],
                        src_ap.bitcast(F32R))
                    xw.append(t)
                xt = [xw[dt // 4][:, (dt % 4) * 512:(dt % 4) * 512 + 512]
                      for dt in range(8)]
                for eb in range(EB):
                    pq = qps.tile([128, 512], F32, tag="q")
                    for dt in range(8):
                        nc.tensor.matmul(
                            pq[:, 0:nw],
                            wq_sb[:, (dt * EB + eb) * 128:(dt * EB + eb + 1) * 128],
                            xt[dt][:, 0:nw], start=(dt == 0), stop=(dt == 7))
                    nc.vector.tensor_scalar_add(
                        qk_sb[:, eb * N + n0:eb * N + n0 + nw],
                        pq[:, 0:nw], bqk_sb[:, eb:eb + 1])
                pv4 = vps.tile([4, 512], F32, tag="v4")
                for dt in range(8):
                    nc.tensor.matmul(pv4[:, 0:nw],
                                     wvsum_sb[:, dt * HPC:(dt + 1) * HPC],
                                     xt[dt][:, 0:nw], start=(dt == 0), stop=(dt == 7))
                nc.vector.tensor_copy(vrow4[:, n0:n0 + nw], pv4[:, 0:nw])

            # ---- phase B: vsum -> per-head columns via DRAM reshape ----
            with tc.tile_pool(name="vdr", bufs=1, space="DRAM") as vdr_pool:
                vdr = vdr_pool.tile([4, N], F32)
                nc.sync.dma_start(vdr[:], vrow4[:])
                for h in range(HPC):
                    nc.sync.dma_start(
                        vcol_all[:, h * NJ:(h + 1) * NJ],
                        vdr[h:h + 1, :].rearrange("o (j p) -> (o p) j", p=128))
                    nc.vector.tensor_scalar_add(
                        vcol_all[:, h * NJ:(h + 1) * NJ],
                        vcol_all[:, h * NJ:(h + 1) * NJ], vbias_sb[:, h:h + 1])

          # ---- phase C2: fold v_spe into V' (in place) ----
          for h in range(HPC):
              for jb in range(NJ):
                  sl = vpT_sb[:, (h * NJ + jb) * 128:(h * NJ + jb + 1) * 128]
                  nc.vector.scalar_tensor_tensor(
                      sl, spe_bcs[h][:], vcol_all[:, h * NJ + jb:h * NJ + jb + 1],
                      sl, MULT, ADD)

        # ---------- phases D+E share outT + wnn ----------
        with tc.tile_pool(name="outp", bufs=1) as outp, \
             tc.tile_pool(name="wnp", bufs=1) as wnp:
            outT_sb = outp.tile([128, HPC * N], F32R)
            wnn_sb = wnp.tile([128, HPC * D], F32R)
            for h in range(HPC):
                nc.sync.dma_start(wnn_sb[:, h * D:(h + 1) * D],
                                  wnn1_d[:, h * D:(h + 1) * D].bitcast(F32R))

            # ---- phase D: attention ----
            with tc.tile_pool(name="pt", bufs=30) as pt_pool, \
                 tc.tile_pool(name="sums", bufs=2) as sum_pool, \
                 tc.tile_pool(name="dps", bufs=4, space="PSUM") as dps, \
                 tc.tile_pool(name="mps", bufs=1, space="PSUM") as mps, \
                 tc.tile_pool(name="ops", bufs=2, space="PSUM") as ops, \
                 tc.tile_pool(name="rps", bufs=1, space="PSUM") as rps:
                for h in range(HPC):
                    qofs, kofs = h * N, (HPC + h) * N
                    for (i0, iw) in CHUNKS:
                        pts = []
                        for jb in range(NJ):
                            pd = dps.tile([128, 512], F32, tag="d")
                            nc.tensor.matmul(pd[:, 0:iw],
                                             qk_sb[:, kofs + jb * 128:kofs + (jb + 1) * 128],
                                             qk_sb[:, qofs + i0:qofs + i0 + iw],
                                             start=True, stop=True)
                            pt = pt_pool.tile([128, 512], BF16, tag="pt")
                            nc.scalar.activation(pt[:, 0:iw], pd[:, 0:iw], AF.Exp,
                                                 scale=SCALE)
                            pts.append(pt)
                        pm = mps.tile([1, 512], F32, tag="m")
                        po = ops.tile([128, 512], F32, tag="o")
                        for jb in range(NJ):
                            nc.tensor.matmul(pm[:, 0:iw], ones_bf[:],
                                             pts[jb][:, 0:iw], start=(jb == 0),
                                             stop=(jb == NJ - 1))
                            nc.tensor.matmul(
                                po[:, 0:iw],
                                vpT_sb[:, (h * NJ + jb) * 128:(h * NJ + jb + 1) * 128],
                                pts[jb][:, 0:iw], start=(jb == 0), stop=(jb == NJ - 1))
                        rsb = sum_pool.tile([1, 512], F32R, tag="r")
                        with nc.allow_low_precision(reason="f32r keeps full fp32 range"):
                            nc.vector.reciprocal(rsb[:, 0:iw], pm[0:1, 0:iw])
                        pr = rps.tile([128, 512], F32, tag="rb")
                        nc.tensor.matmul(pr[:, 0:iw], ones_row[:], rsb[:, 0:iw],
                                         start=True, stop=True)
                        rbs = sum_pool.tile([128, 512], F32, tag="rbs")
                        nc.vector.tensor_copy(rbs[:, 0:iw], pr[:, 0:iw])
                        nc.vector.tensor_tensor(
                            outT_sb[:, h * N + i0:h * N + i0 + iw],
                            po[:, 0:iw], rbs[:, 0:iw], MULT)

            # ---- phase E: nn1 partial + ReduceScatter ----
            with tc.tile_pool(name="fin", bufs=3) as fin_pool, \
                 tc.tile_pool(name="fps", bufs=3, space="PSUM") as fps, \
                 tc.tile_pool(name="dram", bufs=1, space="DRAM") as dram:
                if use_cc:
                    part = dram.tile([D, N], F32)
                    rs0 = dram.tile([CH // 2, N], F32)
                    rs1 = dram.tile([CH // 2, N], F32)
                groups = [[0, 1], [2, 3], [4, 5], [6, 7]]
                for ebo in range(8):
                    fin = fin_pool.tile([128, N], F32, tag="fin")
                    for (n0, nw) in CHUNKS:
                        pf = fps.tile([128, 512], F32, tag="f")
                        for h in range(HPC):
                            nc.tensor.matmul(
                                pf[:, 0:nw],
                                wnn_sb[:, h * D + ebo * 128:h * D + (ebo + 1) * 128],
                                outT_sb[:, h * N + n0:h * N + n0 + nw],
                                start=(h == 0), stop=(h == HPC - 1))
                        nc.scalar.activation(fin[:, n0:n0 + nw], pf[:, 0:nw], AF.Identity,
                                             bias=bnn1_sb[:, ebo:ebo + 1])
                    dst = part if use_cc else out_d
                    nc.sync.dma_start(dst[ebo * 128:(ebo + 1) * 128, :], fin[:])
                    if use_cc and ebo == 3:
                        # overlap first half's pair-reduce with remaining nn1
                        nc.gpsimd.collective_compute(
                            "ReduceScatter", ADD, replica_groups=groups,
                            ins=[part[0:CH, :].opt()], outs=[rs0[:].opt()])
                        nc.sync.dma_start(out_d[0:CH // 2, :], rs0[:])
                if use_cc:
                    nc.gpsimd.collective_compute(
                        "ReduceScatter", ADD, replica_groups=groups,
                        ins=[part[CH:D, :].opt()], outs=[rs1[:].opt()])
                    nc.sync.dma_start(out_d[CH // 2:CH, :], rs1[:])

    nc.compile()
    return nc


def _host_inputs(core, inp):
    b, half = core // 2, core % 2
    h0 = half * HPC
    x = np.asarray(inp["x"][b], dtype=np.float32)            # (D, N)
    Wqkv = np.asarray(inp["Wqkv"], dtype=np.float32)
    bqkv = np.asarray(inp["bqkv"], dtype=np.float32)
    Wspe = np.asarray(inp["Wspe"], dtype=np.float32)[:, :, 0, 0]   # (D, H)
    Wlocal = np.asarray(inp["Wlocal"], dtype=np.float32)     # (D, 8, 3, 3)
    Wnn1 = np.asarray(inp["Wnn1"], dtype=np.float32)
    bnn1 = np.asarray(inp["bnn1"], dtype=np.float32)

    chs = slice(h0 * HD, (h0 + HPC) * HD)                    # this core's 512 channels

    # image layout: reinterpret x^T flat as (D, 48, 48); pad to 50x50
    ximg = np.ascontiguousarray(x.T).reshape(D, N)[chs]      # (512, 2304)
    pad = np.zeros((CH, NPAD), np.float32)
    pad3 = pad[:, :PP * PP].reshape(CH, PP, PP)
    pad3[:, 1:PS + 1, 1:PS + 1] = ximg.reshape(CH, PS, PS)
    xpad = pad

    # qkv weights: e-blocks = [q heads, k heads], lhsT layout; v folded into wvsum
    rows = np.concatenate(
        [np.arange(h0 * HD, (h0 + HPC) * HD) + s * D for s in range(2)])
    wqkvT = Wqkv[rows, :].T                                   # (1024, 1024)
    wq = wqkvT.reshape(8, 128, EB * 128).transpose(1, 0, 2).reshape(128, 8 * EB * 128)
    bqk = bqkv[rows].reshape(8, 128).T.copy()                 # q,k biases (128, 8)
    vrows = np.arange(h0 * HD, (h0 + HPC) * HD) + 2 * D
    wv = Wqkv[vrows, :].reshape(HPC, 128, D).sum(axis=1)      # (HPC, 1024)
    wvsum = wv.T.reshape(8, 128, HPC).transpose(1, 0, 2).reshape(128, 8 * HPC)
    vb = bqkv[vrows].reshape(HPC, 128).sum(axis=1)            # summed v bias per head
    vbias = np.repeat(vb[None, :], 128, axis=0).astype(np.float32)

    # dense per-head conv weights, lhsT[cin, cout] per (head, offset)
    wconv = np.zeros((HPC, 9, 128, 128), np.float32)
    for h in range(HPC):
        for co in range(128):
            g = co // 8
            cg = np.arange(g * 8, g * 8 + 8)
            for oi, (dy, dx) in enumerate(
                    [(a, c) for a in range(3) for c in range(3)]):
                wconv[h, oi, cg, co] = Wlocal[(h0 + h) * HD + co, :, dy, dx]
    wconv = wconv.transpose(2, 0, 1, 3).reshape(128, HPC * 9 * 128)

    # spe block-diag matrix (folds in 1/N pooling mean and attention scale)
    wspe = np.zeros((HPC, 128, 128), np.float32)              # [h, c_in, idx]
    for h in range(HPC):
        for gg in range(16):
            g = (h0 + h) * 16 + gg
            blk = Wspe[g * 8:(g + 1) * 8, :8]                 # [o, i]
            wspe[h, gg * 8:gg * 8 + 8, gg * 8:gg * 8 + 8] = blk.T  # [i, o]
    wspe = (wspe * (SCALE / N)).transpose(1, 0, 2).reshape(128, HPC * 128)

    def fold_bn(g, bta, mu, var):
        s = np.asarray(g, np.float64) / np.sqrt(np.asarray(var, np.float64) + EPS)
        return (s.astype(np.float32),
                (np.asarray(bta, np.float64) - np.asarray(mu, np.float64) * s)
                .astype(np.float32))

    bn_s, bn_b = fold_bn(inp["bn_gamma"], inp["bn_beta"], inp["bn_mean"], inp["bn_var"])
    bnc_s, bnc_b = fold_bn(inp["bnc_gamma"], inp["bnc_beta"], inp["bnc_mean"],
                           inp["bnc_var"])
    shp = lambda a: np.ascontiguousarray(a[chs].reshape(HPC, 128).T)

    wnn1T = Wnn1[:, chs].T                                    # (512, 1024)
    wnn1 = wnn1T.reshape(HPC, 128, D).transpose(1, 0, 2).reshape(128, HPC * D)
    bnn1h = np.ascontiguousarray((0.5 * bnn1).reshape(8, 128).T)

    ones = np.ones((128, 2), np.float32)
    ident = np.eye(128, dtype=np.float32)
    return {
        "x_dn": np.ascontiguousarray(x), "xpad": xpad,
        "wqkv": np.ascontiguousarray(wq), "bqk": np.ascontiguousarray(bqk),
        "wvsum": np.ascontiguousarray(wvsum),
        "vbias": vbias, "wconv": np.ascontiguousarray(wconv),
        "wspe": np.ascontiguousarray(wspe),
        "bn_s": shp(bn_s), "bn_b": shp(bn_b), "bnc_s": shp(bnc_s), "bnc_b": shp(bnc_b),
        "wnn1": np.ascontiguousarray(wnn1), "bnn1h": bnn1h,
        "onesc": ones, "ident": ident,
    }


_NC = None


def kernel(**inputs):
    global _NC
    if _NC is None:
        _NC = _build()
    in_maps = [_host_inputs(c, inputs) for c in range(8)]
    res = run_bass_kernel_spmd(_NC, in_maps, core_ids=list(range(8)))
    out = np.empty((B, N, D), np.float32)
    for b in range(B):
        if USE_COLLECTIVE:
            ev, od = res.results[2 * b]["out"], res.results[2 * b + 1]["out"]
            t = np.empty((D, N), np.float32)
            t[0:256] = ev[0:256]
            t[256:512] = od[0:256]
            t[512:768] = ev[256:512]
            t[768:1024] = od[256:512]
        else:
            t = res.results[2 * b]["out"] + res.results[2 * b + 1]["out"]
        out[b] = t.T
    return out


def run_timed(**inputs):
    """Re-run with NTFF tracing to get HW exec time (best effort)."""
    global _NC
    if _NC is None:
        _NC = _build()
    in_maps = [_host_inputs(c, inputs) for c in range(8)]
    try:
        return run_bass_kernel_spmd(_NC, in_maps, core_ids=list(range(8)), trace=True)
    except Exception as e:  # tracing unsupported under some axon terminals
        print(f"trace run failed: {e}")
        return None



# revision 7
# speedup vs baseline: 1.0714x; 1.0714x over previous
"""Trainium2 Bass kernel for nn_Attention_87857851006980.

Sharding: 8 cores = 4 batches x 2 head-halves. Core c handles batch c//2,
heads [0..4) (even c) or [4..8) (odd c). Each core computes qkv for its
heads (full-d contraction), the conv/spe branches for its heads' channels,
attention for its heads, and a partial nn1 over its 512 channels; a
pair-wise ReduceScatter then sums the nn1 partials, leaving each core with
half of the output channels for its batch. Host gathers/transposes.

All heavy math runs on-device; the host only slices/transposes inputs
(layout) and folds BN/bias constants into weight tensors.
"""
import sys
sys.path.insert(0, "/opt/trn_rl_repo")
import numpy as np
import ml_dtypes

import concourse.bacc as bacc
import concourse.bass as bass
import concourse.tile as tile
import concourse.mybir as mybir
from concourse.bass_utils import run_bass_kernel_spmd

F32 = mybir.dt.float32
F32R = mybir.dt.float32r
BF16 = mybir.dt.bfloat16
AF = mybir.ActivationFunctionType
ADD = mybir.AluOpType.add
MULT = mybir.AluOpType.mult

B, D, N, H, HD = 4, 1024, 2304, 8, 128
PS = 48          # image side; N = PS*PS
PP = PS + 2      # padded side
NPAD = PP * PP + 2  # 2502: +2 so the (dy,dx)=(1,1) view of the last row-chunk stays in bounds
HPC = 4          # heads per core
CH = HPC * HD    # 512 channels per core
EPS = 1e-5
SCALE = D ** -0.5

# n-chunks of the free dimension (all >=256 so f32r runs at full rate)
CHUNKS = [(0, 512), (512, 512), (1024, 512), (1536, 512), (2048, 256)]
NJ = N // 128    # 18 key blocks
EB = 8           # qkv output blocks per core: 4 q + 4 k (v folded into wvsum)

USE_COLLECTIVE = True


def _build(single=False):
    use_cc = USE_COLLECTIVE and not single
    nc = bacc.Bacc("TRN2", target_bir_lowering=False, debug=False,
                   num_devices=1 if single else 8)

    # ---- DRAM I/O ----
    x_dn = nc.dram_tensor("x_dn", [D, N], F32, kind="ExternalInput").ap()
    xpad_d = nc.dram_tensor("xpad", [CH, NPAD], F32, kind="ExternalInput").ap()
    wqkv_d = nc.dram_tensor("wqkv", [128, 8 * EB * 128], F32, kind="ExternalInput").ap()
    wvsum_d = nc.dram_tensor("wvsum", [128, 8 * HPC], F32, kind="ExternalInput").ap()
    bqk_d = nc.dram_tensor("bqk", [128, 8], F32, kind="ExternalInput").ap()
    vbias_d = nc.dram_tensor("vbias", [128, HPC], F32, kind="ExternalInput").ap()
    wconv_d = nc.dram_tensor("wconv", [128, HPC * 9 * 128], F32, kind="ExternalInput").ap()
    wspe_d = nc.dram_tensor("wspe", [128, HPC * 128], F32, kind="ExternalInput").ap()
    bn_s_d = nc.dram_tensor("bn_s", [128, HPC], F32, kind="ExternalInput").ap()
    bn_b_d = nc.dram_tensor("bn_b", [128, HPC], F32, kind="ExternalInput").ap()
    bnc_s_d = nc.dram_tensor("bnc_s", [128, HPC], F32, kind="ExternalInput").ap()
    bnc_b_d = nc.dram_tensor("bnc_b", [128, HPC], F32, kind="ExternalInput").ap()
    wnn1_d = nc.dram_tensor("wnn1", [128, HPC * D], BF16, kind="ExternalInput").ap()
    bnn1_d = nc.dram_tensor("bnn1h", [128, 8], F32, kind="ExternalInput").ap()
    ones_d = nc.dram_tensor("onesc", [128, 2], F32, kind="ExternalInput").ap()
    ident_d = nc.dram_tensor("ident", [128, 128], F32, kind="ExternalInput").ap()
    if use_cc:
        out_d = nc.dram_tensor("out", [CH, N], F32, kind="ExternalOutput").ap()
    else:
        out_d = nc.dram_tensor("out", [D, N], F32, kind="ExternalOutput").ap()

    with tile.TileContext(nc) as tc:
      with tc.tile_pool(name="persist", bufs=1) as pp:
        # ---------- persistent tiles ----------
        qk_sb = pp.tile([128, 8 * N], BF16, tag="qk")   # q then k, 4 heads each
        vpT_sb = pp.tile([128, HPC * NJ * 128], BF16, tag="vpT")  # V' (cbr^T, then +v_spe)
        vcol_all = pp.tile([128, HPC * NJ], F32, tag="vcol_all")
        spe_row = pp.tile([1, HPC * 128], F32R, tag="spe_row")
        ones_sb = pp.tile([128, 2], F32R, tag="ones")
        ones_bf = pp.tile([128, 1], BF16, tag="ones_bf")
        ones_row = pp.tile([1, 128], F32R, tag="ones_row")
        ident_sb = pp.tile([128, 128], F32, tag="ident")
        bqk_sb = pp.tile([128, 8], F32, tag="bqk")
        vbias_sb = pp.tile([128, HPC], F32, tag="vbias")
        bn_s = pp.tile([128, HPC], F32, tag="bn_s")
        bn_b = pp.tile([128, HPC], F32, tag="bn_b")
        bnc_s = pp.tile([128, HPC], F32, tag="bnc_s")
        bnc_b = pp.tile([128, HPC], F32, tag="bnc_b")
        bnn1_sb = pp.tile([128, 8], F32, tag="bnn1")
        wspe_sb = pp.tile([128, HPC * 128], F32R, tag="wspe")
        wvsum_sb = pp.tile([128, 8 * HPC], F32R, tag="wvsum")
        ident_bf = pp.tile([128, 128], BF16, tag="ident_bf")

        # spe_bc tiles live from phase C1 until the C2 fold
        with tc.tile_pool(name="spb", bufs=4) as spb_pool:
          spe_bcs = []

          # ---- phase C1: conv + spe; vpT := cbr^T ----
          with tc.tile_pool(name="cvin", bufs=2) as cvin_pool, \
               tc.tile_pool(name="cvw", bufs=2) as cvw_pool, \
               tc.tile_pool(name="cbr", bufs=2) as cbr_pool, \
               tc.tile_pool(name="scr", bufs=1) as scr_pool, \
               tc.tile_pool(name="pcol", bufs=2) as pcol_pool, \
               tc.tile_pool(name="cps", bufs=2, space="PSUM") as cps, \
               tc.tile_pool(name="tps", bufs=3, space="PSUM") as tps, \
               tc.tile_pool(name="sps", bufs=1, space="PSUM") as sps:
            xp0 = cvin_pool.tile([128, NPAD], F32R, tag="xp")
            nc.sync.dma_start(xp0[:], xpad_d[0:128, :].bitcast(F32R))
            wcv0 = cvw_pool.tile([128, 9 * 128], F32R, tag="wcv")
            nc.sync.dma_start(wcv0[:], wconv_d[:, 0:9 * 128].bitcast(F32R))
            nc.sync.dma_start(ones_sb[:], ones_d[:].bitcast(F32R))
            nc.vector.tensor_copy(ones_bf[:], ones_sb[:, 0:1].bitcast(F32))
            nc.sync.dma_start(ones_row[:], ones_sb[:, 0:1])
            nc.sync.dma_start(ident_sb[:], ident_d[:])
            nc.sync.dma_start(bqk_sb[:], bqk_d[:])
            nc.sync.dma_start(vbias_sb[:], vbias_d[:])
            nc.sync.dma_start(bn_s[:], bn_s_d[:])
            nc.sync.dma_start(bn_b[:], bn_b_d[:])
            nc.sync.dma_start(bnc_s[:], bnc_s_d[:])
            nc.sync.dma_start(bnc_b[:], bnc_b_d[:])
            nc.sync.dma_start(bnn1_sb[:], bnn1_d[:])
            nc.sync.dma_start(wspe_sb[:], wspe_d[:].bitcast(F32R))
            nc.sync.dma_start(wvsum_sb[:], wvsum_d[:].bitcast(F32R))
            nc.vector.tensor_copy(ident_bf[:], ident_sb[:])

            for h in range(HPC):
                if h == 0:
                    xp, wcv = xp0, wcv0
                else:
                    xp = cvin_pool.tile([128, NPAD], F32R, tag="xp")
                    nc.sync.dma_start(xp[:],
                                      xpad_d[h * 128:(h + 1) * 128, :].bitcast(F32R))
                    wcv = cvw_pool.tile([128, 9 * 128], F32R, tag="wcv")
                    nc.sync.dma_start(
                        wcv[:], wconv_d[:, h * 9 * 128:(h + 1) * 9 * 128].bitcast(F32R))

                # spe branch: gelu(bn(x)) with running row-sum -> pooled -> spe row
                scr = scr_pool.tile([128, N], BF16, tag="scr")
                pcol = pcol_pool.tile([128, 1], F32, tag="pcol")
                interior = xp[:, PP + 1:PP + 1 + PS * PP].rearrange(
                    "p (r c) -> p r c", c=PP)[:, :, 0:PS].bitcast(F32)
                nc.scalar.activation(
                    scr[:].rearrange("p (r c) -> p r c", c=PS), interior,
                    AF.Gelu, bias=bn_b[:, h:h + 1], scale=bn_s[:, h:h + 1],
                    accum_out=pcol[:],
                )
                pcol_r = pcol_pool.tile([128, 1], F32R, tag="pcolr")
                nc.vector.tensor_copy(pcol_r[:], pcol[:])
                ps_spe = sps.tile([1, 128], F32, tag="spe")
                nc.tensor.matmul(ps_spe[:], pcol_r[:],
                                 wspe_sb[:, h * 128:(h + 1) * 128], start=True, stop=True)
                nc.vector.tensor_copy(spe_row[:, h * 128:(h + 1) * 128], ps_spe[:])
                # broadcast spe over partitions: spe_bc[p, c] = spe[c]
                ps_bc = sps.tile([128, 128], F32, tag="bc")
                nc.tensor.matmul(ps_bc[:], ones_row[:],
                                 spe_row[:, h * 128:(h + 1) * 128], start=True, stop=True)
                spe_bc = spb_pool.tile([128, 128], F32, tag="spb")
                nc.vector.tensor_copy(spe_bc[:], ps_bc[:])
                spe_bcs.append(spe_bc)

                # conv branch: 9 shifted matmuls, bn+gelu
                cbr = cbr_pool.tile([128, N], BF16, tag="cbr")
                for rc in range(6):  # 8 output rows per chunk
                    r0 = rc * 8
                    pc = cps.tile([128, 8 * PS], F32, tag="cv")
                    for oi, (dy, dx) in enumerate(
                            [(a, b) for a in (-1, 0, 1) for b in (-1, 0, 1)]):
                        base = (r0 + 1 + dy) * PP + 1 + dx
                        rhs = xp[:, base:base + 8 * PP].rearrange(
                            "p (r c) -> p r c", c=PP)[:, :, 0:PS]
                        nc.tensor.matmul(pc[:].rearrange("p (r c) -> p r c", c=PS),
                                         wcv[:, oi * 128:(oi + 1) * 128], rhs,
                                         start=(oi == 0), stop=(oi == 8))
                    nc.scalar.activation(cbr[:, r0 * PS:(r0 + 8) * PS], pc[:],
                                         AF.Gelu, bias=bnc_b[:, h:h + 1],
                                         scale=bnc_s[:, h:h + 1])
                for jb in range(NJ):
                    pt = tps.tile([128, 128], BF16, tag="tp")
                    nc.tensor.transpose(pt[:], cbr[:, jb * 128:(jb + 1) * 128], ident_bf[:])
                    nc.vector.tensor_copy(
                        vpT_sb[:, (h * NJ + jb) * 128:(h * NJ + jb + 1) * 128], pt[:])

          # ---- phase A: qkv projection (+ folded vsum rows) ----
          with tc.tile_pool(name="wqp", bufs=1) as wqp, \
               tc.tile_pool(name="vr", bufs=1) as vr_pool, \
               tc.tile_pool(name="xch", bufs=4) as xch_pool, \
               tc.tile_pool(name="qps", bufs=4, space="PSUM") as qps, \
               tc.tile_pool(name="vps", bufs=2, space="PSUM") as vps:
            wq_sb = wqp.tile([128, 8 * EB * 128], F32R)
            for dt in range(8):
                nc.sync.dma_start(
                    wq_sb[:, dt * EB * 128:(dt + 1) * EB * 128],
                    wqkv_d[:, dt * EB * 128:(dt + 1) * EB * 128].bitcast(F32R))
            vrow4 = vr_pool.tile([4, N], F32)
            for (n0, nw) in CHUNKS:
                xw = []
                for half in range(2):
                    t = xch_pool.tile([128, 4 * 512], F32R, tag="x")
                    src_ap = x_dn[half * 512:(half + 1) * 512, n0:n0 + nw].rearrange(
                        "(t p) n -> p t n", p=128)
                    nc.sync.dma_start(
                        t[:].rearrange("p (t n) -> p t n", t=4)[:, :, 0:nw],
                        src_ap.bitcast(F32R))
                    xw.append(t)
                xt = [xw[dt // 4][:, (dt % 4) * 512:(dt % 4) * 512 + 512]
                      for dt in range(8)]
                for eb in range(EB):
                    pq = qps.tile([128, 512], F32, tag="q")
                    for dt in range(8):
                        nc.tensor.matmul(
                            pq[:, 0:nw],
                            wq_sb[:, (dt * EB + eb) * 128:(dt * EB + eb + 1) * 128],
                            xt[dt][:, 0:nw], start=(dt == 0), stop=(dt == 7))
                    nc.vector.tensor_scalar_add(
                        qk_sb[:, eb * N + n0:eb * N + n0 + nw],
                        pq[:, 0:nw], bqk_sb[:, eb:eb + 1])
                pv4 = vps.tile([4, 512], F32, tag="v4")
                for dt in range(8):
                    nc.tensor.matmul(pv4[:, 0:nw],
                                     wvsum_sb[:, dt * HPC:(dt + 1) * HPC],
                                     xt[dt][:, 0:nw], start=(dt == 0), stop=(dt == 7))
                nc.vector.tensor_copy(vrow4[:, n0:n0 + nw], pv4[:, 0:nw])

            # ---- phase B: vsum -> per-head columns via DRAM reshape ----
            with tc.tile_pool(name="vdr", bufs=1, space="DRAM") as vdr_pool:
                vdr = vdr_pool.tile([4, N], F32)
                nc.sync.dma_start(vdr[:], vrow4[:])
                for h in range(HPC):
                    nc.sync.dma_start(
                        vcol_all[:, h * NJ:(h + 1) * NJ],
                        vdr[h:h + 1, :].rearrange("o (j p) -> (o p) j", p=128))
                    nc.vector.tensor_scalar_add(
                        vcol_all[:, h * NJ:(h + 1) * NJ],
                        vcol_all[:, h * NJ:(h + 1) * NJ], vbias_sb[:, h:h + 1])

          # ---- phase C2: fold v_spe into V' (in place) ----
          for h in range(HPC):
              for jb in range(NJ):
                  sl = vpT_sb[:, (h * NJ + jb) * 128:(h * NJ + jb + 1) * 128]
                  nc.vector.scalar_tensor_tensor(
                      sl, spe_bcs[h][:], vcol_all[:, h * NJ + jb:h * NJ + jb + 1],
                      sl, MULT, ADD)

        # ---------- phases D+E share outT + wnn ----------
        with tc.tile_pool(name="outp", bufs=1) as outp, \
             tc.tile_pool(name="wnp", bufs=1) as wnp:
            outT_sb = outp.tile([128, HPC * N], BF16)
            wnn_sb = wnp.tile([128, HPC * D], BF16)
            for h in range(HPC):
                nc.sync.dma_start(wnn_sb[:, h * D:(h + 1) * D],
                                  wnn1_d[:, h * D:(h + 1) * D])

            # ---- phase D: attention ----
            # exp(dots) lands in one contiguous [128, NJ*512] bf16 tile per
            # (h, chunk); the softmax denominator is a DVE add-tree over the
            # NJ blocks plus one ones-matmul (instead of NJ PE matmuls).
            with tc.tile_pool(name="pt", bufs=2) as pt_pool, \
                 tc.tile_pool(name="str", bufs=2) as s_pool, \
                 tc.tile_pool(name="sums", bufs=2) as sum_pool, \
                 tc.tile_pool(name="dps", bufs=2, space="PSUM") as dps, \
                 tc.tile_pool(name="mps", bufs=1, space="PSUM") as mps, \
                 tc.tile_pool(name="ops", bufs=2, space="PSUM") as ops, \
                 tc.tile_pool(name="rps", bufs=1, space="PSUM") as rps:
                for h in range(HPC):
                    qofs, kofs = h * N, (HPC + h) * N
                    for (i0, iw) in CHUNKS:
                        pt_all = pt_pool.tile([128, NJ * 512], BF16, tag="pt")
                        if iw == 512:
                            for jp in range(NJ // 2):
                                pd = dps.tile([128, 1024], F32, tag="d")
                                for e in range(2):
                                    jb = 2 * jp + e
                                    nc.tensor.matmul(
                                        pd[:, e * 512:e * 512 + iw],
                                        qk_sb[:, kofs + jb * 128:kofs + (jb + 1) * 128],
                                        qk_sb[:, qofs + i0:qofs + i0 + iw],
                                        start=True, stop=True)
                                nc.scalar.activation(
                                    pt_all[:, jp * 1024:(jp + 1) * 1024],
                                    pd[:], AF.Exp, scale=SCALE)
                        else:
                            for jp in range(NJ // 2):
                                pd = dps.tile([128, 1024], F32, tag="d")
                                for e in range(2):
                                    jb = 2 * jp + e
                                    nc.tensor.matmul(
                                        pd[:, e * 512:e * 512 + iw],
                                        qk_sb[:, kofs + jb * 128:kofs + (jb + 1) * 128],
                                        qk_sb[:, qofs + i0:qofs + i0 + iw],
                                        start=True, stop=True)
                                    nc.scalar.activation(
                                        pt_all[:, jb * 512:jb * 512 + iw],
                                        pd[:, e * 512:e * 512 + iw], AF.Exp,
                                        scale=SCALE)
                        po = ops.tile([128, 512], F32, tag="o")
                        for jb in range(NJ):
                            nc.tensor.matmul(
                                po[:, 0:iw],
                                vpT_sb[:, (h * NJ + jb) * 128:(h * NJ + jb + 1) * 128],
                                pt_all[:, jb * 512:jb * 512 + iw],
                                start=(jb == 0), stop=(jb == NJ - 1))
                        # denominator: 18-block sum as a DVE tree into S
                        S = s_pool.tile([128, 9 * 512], BF16, tag="S")
                        if iw == 512:
                            nc.vector.tensor_tensor(
                                S[:], pt_all[:, 0:9 * 512],
                                pt_all[:, 9 * 512:18 * 512], ADD)
                            nc.vector.tensor_tensor(
                                S[:, 0:4 * 512], S[:, 0:4 * 512],
                                S[:, 4 * 512:8 * 512], ADD)
                            nc.vector.tensor_tensor(
                                S[:, 0:2 * 512], S[:, 0:2 * 512],
                                S[:, 2 * 512:4 * 512], ADD)
                            nc.vector.tensor_tensor(
                                S[:, 0:512], S[:, 0:512], S[:, 512:1024], ADD)
                            nc.vector.tensor_tensor(
                                S[:, 0:512], S[:, 0:512],
                                S[:, 8 * 512:9 * 512], ADD)
                        else:
                            ptv = pt_all[:].rearrange("p (n w) -> p n w", w=512)
                            Sv = S[:].rearrange("p (n w) -> p n w", w=512)
                            nc.vector.tensor_tensor(
                                Sv[:, 0:9, 0:iw], ptv[:, 0:9, 0:iw],
                                ptv[:, 9:18, 0:iw], ADD)
                            nc.vector.tensor_tensor(
                                Sv[:, 0:4, 0:iw], Sv[:, 0:4, 0:iw],
                                Sv[:, 4:8, 0:iw], ADD)
                            nc.vector.tensor_tensor(
                                Sv[:, 0:2, 0:iw], Sv[:, 0:2, 0:iw],
                                Sv[:, 2:4, 0:iw], ADD)
                            nc.vector.tensor_tensor(
                                Sv[:, 0:1, 0:iw], Sv[:, 0:1, 0:iw],
                                Sv[:, 1:2, 0:iw], ADD)
                            nc.vector.tensor_tensor(
                                Sv[:, 0:1, 0:iw], Sv[:, 0:1, 0:iw],
                                Sv[:, 8:9, 0:iw], ADD)
                        pm = mps.tile([1, 512], F32, tag="m")
                        nc.tensor.matmul(pm[:, 0:iw], ones_bf[:], S[:, 0:iw],
                                         start=True, stop=True)
                        rsb = sum_pool.tile([1, 512], F32R, tag="r")
                        with nc.allow_low_precision(reason="f32r keeps full fp32 range"):
                            nc.vector.reciprocal(rsb[:, 0:iw], pm[0:1, 0:iw])
                        pr = rps.tile([128, 512], F32, tag="rb")
                        nc.tensor.matmul(pr[:, 0:iw], ones_row[:], rsb[:, 0:iw],
                                         start=True, stop=True)
                        rbs = sum_pool.tile([128, 512], F32, tag="rbs")
                        nc.vector.tensor_copy(rbs[:, 0:iw], pr[:, 0:iw])
                        nc.vector.tensor_tensor(
                            outT_sb[:, h * N + i0:h * N + i0 + iw],
                            po[:, 0:iw], rbs[:, 0:iw], MULT)

            # ---- phase E: nn1 partial + ReduceScatter ----
            with tc.tile_pool(name="fin", bufs=3) as fin_pool, \
                 tc.tile_pool(name="fps", bufs=3, space="PSUM") as fps, \
                 tc.tile_pool(name="dram", bufs=1, space="DRAM") as dram:
                if use_cc:
                    part = dram.tile([D, N], F32)
                    rs0 = dram.tile([CH // 2, N], F32)
                    rs1 = dram.tile([CH // 2, N], F32)
                groups = [[0, 1], [2, 3], [4, 5], [6, 7]]
                for ebo in range(8):
                    fin = fin_pool.tile([128, N], F32, tag="fin")
                    for (n0, nw) in CHUNKS:
                        pf = fps.tile([128, 512], F32, tag="f")
                        for h in range(HPC):
                            nc.tensor.matmul(
                                pf[:, 0:nw],
                                wnn_sb[:, h * D + ebo * 128:h * D + (ebo + 1) * 128],
                                outT_sb[:, h * N + n0:h * N + n0 + nw],
                                start=(h == 0), stop=(h == HPC - 1))
                        nc.scalar.activation(fin[:, n0:n0 + nw], pf[:, 0:nw], AF.Identity,
                                             bias=bnn1_sb[:, ebo:ebo + 1])
                    dst = part if use_cc else out_d
                    nc.sync.dma_start(dst[ebo * 128:(ebo + 1) * 128, :], fin[:])
                    if use_cc and ebo == 3:
                        # overlap first half's pair-reduce with remaining nn1
                        nc.gpsimd.collective_compute(
                            "ReduceScatter", ADD, replica_groups=groups,
                            ins=[part[0:CH, :].opt()], outs=[rs0[:].opt()])
                        nc.sync.dma_start(out_d[0:CH // 2, :], rs0[:])
                if use_cc:
                    nc.gpsimd.collective_compute(
                        "ReduceScatter", ADD, replica_groups=groups,
                        ins=[part[CH:D, :].opt()], outs=[rs1[:].opt()])
                    nc.sync.dma_start(out_d[CH // 2:CH, :], rs1[:])

    nc.compile()
    return nc


def _host_inputs(core, inp):
    b, half = core // 2, core % 2
    h0 = half * HPC
    x = np.asarray(inp["x"][b], dtype=np.float32)            # (D, N)
    Wqkv = np.asarray(inp["Wqkv"], dtype=np.float32)
    bqkv = np.asarray(inp["bqkv"], dtype=np.float32)
    Wspe = np.asarray(inp["Wspe"], dtype=np.float32)[:, :, 0, 0]   # (D, H)
    Wlocal = np.asarray(inp["Wlocal"], dtype=np.float32)     # (D, 8, 3, 3)
    Wnn1 = np.asarray(inp["Wnn1"], dtype=np.float32)
    bnn1 = np.asarray(inp["bnn1"], dtype=np.float32)

    chs = slice(h0 * HD, (h0 + HPC) * HD)                    # this core's 512 channels

    # image layout: reinterpret x^T flat as (D, 48, 48); pad to 50x50
    ximg = np.ascontiguousarray(x.T).reshape(D, N)[chs]      # (512, 2304)
    pad = np.zeros((CH, NPAD), np.float32)
    pad3 = pad[:, :PP * PP].reshape(CH, PP, PP)
    pad3[:, 1:PS + 1, 1:PS + 1] = ximg.reshape(CH, PS, PS)
    xpad = pad

    # qkv weights: e-blocks = [q heads, k heads], lhsT layout; v folded into wvsum
    rows = np.concatenate(
        [np.arange(h0 * HD, (h0 + HPC) * HD) + s * D for s in range(2)])
    wqkvT = Wqkv[rows, :].T                                   # (1024, 1024)
    wq = wqkvT.reshape(8, 128, EB * 128).transpose(1, 0, 2).reshape(128, 8 * EB * 128)
    bqk = bqkv[rows].reshape(8, 128).T.copy()                 # q,k biases (128, 8)
    vrows = np.arange(h0 * HD, (h0 + HPC) * HD) + 2 * D
    wv = Wqkv[vrows, :].reshape(HPC, 128, D).sum(axis=1)      # (HPC, 1024)
    wvsum = wv.T.reshape(8, 128, HPC).transpose(1, 0, 2).reshape(128, 8 * HPC)
    vb = bqkv[vrows].reshape(HPC, 128).sum(axis=1)            # summed v bias per head
    vbias = np.repeat(vb[None, :], 128, axis=0).astype(np.float32)

    # dense per-head conv weights, lhsT[cin, cout] per (head, offset)
    wconv = np.zeros((HPC, 9, 128, 128), np.float32)
    for h in range(HPC):
        for co in range(128):
            g = co // 8
            cg = np.arange(g * 8, g * 8 + 8)
            for oi, (dy, dx) in enumerate(
                    [(a, c) for a in range(3) for c in range(3)]):
                wconv[h, oi, cg, co] = Wlocal[(h0 + h) * HD + co, :, dy, dx]
    wconv = wconv.transpose(2, 0, 1, 3).reshape(128, HPC * 9 * 128)

    # spe block-diag matrix (folds in 1/N pooling mean and attention scale)
    wspe = np.zeros((HPC, 128, 128), np.float32)              # [h, c_in, idx]
    for h in range(HPC):
        for gg in range(16):
            g = (h0 + h) * 16 + gg
            blk = Wspe[g * 8:(g + 1) * 8, :8]                 # [o, i]
            wspe[h, gg * 8:gg * 8 + 8, gg * 8:gg * 8 + 8] = blk.T  # [i, o]
    wspe = (wspe * (SCALE / N)).transpose(1, 0, 2).reshape(128, HPC * 128)

    def fold_bn(g, bta, mu, var):
        s = np.asarray(g, np.float64) / np.sqrt(np.asarray(var, np.float64) + EPS)
        return (s.astype(np.float32),
                (np.asarray(bta, np.float64) - np.asarray(mu, np.float64) * s)
                .astype(np.float32))

    bn_s, bn_b = fold_bn(inp["bn_gamma"], inp["bn_beta"], inp["bn_mean"], inp["bn_var"])
    bnc_s, bnc_b = fold_bn(inp["bnc_gamma"], inp["bnc_beta"], inp["bnc_mean"],
                           inp["bnc_var"])
    shp = lambda a: np.ascontiguousarray(a[chs].reshape(HPC, 128).T)

    wnn1T = Wnn1[:, chs].T                                    # (512, 1024)
    wnn1 = wnn1T.reshape(HPC, 128, D).transpose(1, 0, 2).reshape(128, HPC * D)
    wnn1 = wnn1.astype(ml_dtypes.bfloat16)
    bnn1h = np.ascontiguousarray((0.5 * bnn1).reshape(8, 128).T)

    ones = np.ones((128, 2), np.float32)
    ident = np.eye(128, dtype=np.float32)
    return {
        "x_dn": np.ascontiguousarray(x), "xpad": xpad,
        "wqkv": np.ascontiguousarray(wq), "bqk": np.ascontiguousarray(bqk),
        "wvsum": np.ascontiguousarray(wvsum),
        "vbias": vbias, "wconv": np.ascontiguousarray(wconv),
        "wspe": np.ascontiguousarray(wspe),
        "bn_s": shp(bn_s), "bn_b": shp(bn_b), "bnc_s": shp(bnc_s), "bnc_b": shp(bnc_b),
        "wnn1": np.ascontiguousarray(wnn1), "bnn1h": bnn1h,
        "onesc": ones, "ident": ident,
    }


_NC = None


def kernel(**inputs):
    global _NC
    if _NC is None:
        _NC = _build()
    in_maps = [_host_inputs(c, inputs) for c in range(8)]
    res = run_bass_kernel_spmd(_NC, in_maps, core_ids=list(range(8)))
    out = np.empty((B, N, D), np.float32)
    for b in range(B):
        if USE_COLLECTIVE:
            ev, od = res.results[2 * b]["out"], res.results[2 * b + 1]["out"]
            t = np.empty((D, N), np.float32)
            t[0:256] = ev[0:256]
            t[256:512] = od[0:256]
            t[512:768] = ev[256:512]
            t[768:1024] = od[256:512]
        else:
            t = res.results[2 * b]["out"] + res.results[2 * b + 1]["out"]
        out[b] = t.T
    return out


def run_timed(**inputs):
    """Re-run with NTFF tracing to get HW exec time (best effort)."""
    global _NC
    if _NC is None:
        _NC = _build()
    in_maps = [_host_inputs(c, inputs) for c in range(8)]
    try:
        return run_bass_kernel_spmd(_NC, in_maps, core_ids=list(range(8)), trace=True)
    except Exception as e:  # tracing unsupported under some axon terminals
        print(f"trace run failed: {e}")
        return None



# revision 31
# speedup vs baseline: 1.3245x; 1.2363x over previous
"""Trainium2 Bass kernel for nn_Attention_87857851006980.

Sharding: 8 cores = 4 batches x 2 head-halves. Core c handles batch c//2,
heads [0..4) (even c) or [4..8) (odd c). Each core computes qkv for its
heads (full-d contraction), the conv/spe branches for its heads' channels,
attention for its heads, and a partial nn1 over its 512 channels; a
pair-wise ReduceScatter then sums the nn1 partials, leaving each core with
half of the output channels for its batch. Host gathers/transposes.

fp8 DoubleRow (2 MACs/cycle) is used for the qkv projection, the grouped
3x3 conv and the q@k dots; attention@V and nn1 stay bf16. The softmax
denominator is computed as a DVE add-tree over the exp blocks plus one
ones-matmul instead of NJ extra PE matmuls.
"""
import sys
sys.path.insert(0, "/opt/trn_rl_repo")
import numpy as np
import ml_dtypes

import concourse.bacc as bacc
import concourse.bass as bass
import concourse.tile as tile
import concourse.mybir as mybir
from concourse.bass_utils import run_bass_kernel_spmd

F32 = mybir.dt.float32
F32R = mybir.dt.float32r
BF16 = mybir.dt.bfloat16
FP8 = mybir.dt.float8e4
AF = mybir.ActivationFunctionType
ADD = mybir.AluOpType.add
MULT = mybir.AluOpType.mult
DRM = mybir.MatmulPerfMode.DoubleRow

B, D, N, H, HD = 4, 1024, 2304, 8, 128
PS = 48          # image side; N = PS*PS
PP = PS + 2      # padded side
NPAD = PP * PP + 2  # 2502: +2 so the (dy,dx)=(1,1) view of the last row-chunk stays in bounds
HPC = 4          # heads per core
CH = HPC * HD    # 512 channels per core
EPS = 1e-5
SCALE = D ** -0.5

CHUNKS = [(0, 512), (512, 512), (1024, 512), (1536, 512), (2048, 256)]
NJ = N // 128    # 18 key blocks
EB = 8           # qkv output blocks per core: 4 q + 4 k (v folded into wvsum)

USE_COLLECTIVE = True


def _build(single=False, dump=False):
    use_cc = USE_COLLECTIVE and not single
    nc = bacc.Bacc("TRN2", target_bir_lowering=False, debug=False,
                   num_devices=1 if single else 8)
    if dump:
        dbg_qk_d = nc.dram_tensor("dbg_qk", [64, EB * 2 * N], FP8,
                                  kind="ExternalOutput").ap()
        dbg_vpT_d = nc.dram_tensor("dbg_vpT", [128, HPC * NJ * 128], BF16,
                                   kind="ExternalOutput").ap()
        dbg_outT_d = nc.dram_tensor("dbg_outT", [128, HPC * N], BF16,
                                    kind="ExternalOutput").ap()
        dbg_vcol_d = nc.dram_tensor("dbg_vcol", [128, HPC * NJ], F32,
                                    kind="ExternalOutput").ap()
        dbg_spe_d = nc.dram_tensor("dbg_spe", [1, HPC * 128], F32,
                                   kind="ExternalOutput").ap()

    # ---- DRAM I/O ----
    x8_d = nc.dram_tensor("x8", [64, 8 * 2 * N], FP8, kind="ExternalInput").ap()
    xspe_d = nc.dram_tensor("xspe", [128, HPC * N], BF16, kind="ExternalInput").ap()
    xpad8_d = nc.dram_tensor("xpad8", [64, HPC * 2 * NPAD], FP8, kind="ExternalInput").ap()
    wq8_d = nc.dram_tensor("wq8", [64, 8 * EB * 2 * 128], FP8, kind="ExternalInput").ap()
    wv8_d = nc.dram_tensor("wv8", [64, 8 * 2 * 16], FP8, kind="ExternalInput").ap()
    bqk_d = nc.dram_tensor("bqk", [128, 8], F32, kind="ExternalInput").ap()
    vbias_d = nc.dram_tensor("vbias", [128, HPC], F32, kind="ExternalInput").ap()
    wcv8_d = nc.dram_tensor("wcv8", [64, HPC * 9 * 2 * 128], FP8, kind="ExternalInput").ap()
    wspe_d = nc.dram_tensor("wspe", [128, HPC * 128], F32, kind="ExternalInput").ap()
    bn_s_d = nc.dram_tensor("bn_s", [128, HPC], F32, kind="ExternalInput").ap()
    bn_b_d = nc.dram_tensor("bn_b", [128, HPC], F32, kind="ExternalInput").ap()
    bnc_s_d = nc.dram_tensor("bnc_s", [128, HPC], F32, kind="ExternalInput").ap()
    bnc_b_d = nc.dram_tensor("bnc_b", [128, HPC], F32, kind="ExternalInput").ap()
    wnn1_d = nc.dram_tensor("wnn1", [128, HPC * D], BF16, kind="ExternalInput").ap()
    bnn1_d = nc.dram_tensor("bnn1h", [128, 8], F32, kind="ExternalInput").ap()
    ones_d = nc.dram_tensor("onesc", [128, 2], F32, kind="ExternalInput").ap()
    ident_d = nc.dram_tensor("ident", [128, 128], F32, kind="ExternalInput").ap()
    if use_cc:
        out_d = nc.dram_tensor("out", [CH, N], F32, kind="ExternalOutput").ap()
    else:
        out_d = nc.dram_tensor("out", [D, N], F32, kind="ExternalOutput").ap()

    with tile.TileContext(nc) as tc:
      with tc.tile_pool(name="persist", bufs=1) as pp:
        # ---------- persistent tiles ----------
        qk8_sb = pp.tile([64, EB * 2 * N], FP8, tag="qk8")  # q then k heads, d-pair DR layout
        vpT_sb = pp.tile([128, HPC * NJ * 128], BF16, tag="vpT")  # V' (cbr^T, then +v_spe)
        outT_sb = pp.tile([128, HPC * N], BF16, tag="outT")
        wnn_sb = pp.tile([128, HPC * D], BF16, tag="wnn")
        vcol_all = pp.tile([128, HPC * NJ], F32, tag="vcol_all")
        spe_row = pp.tile([1, HPC * 128], F32R, tag="spe_row")
        ones_sb = pp.tile([128, 2], F32R, tag="ones")
        ones_bf = pp.tile([128, 1], BF16, tag="ones_bf")
        ones_row = pp.tile([1, 128], F32R, tag="ones_row")
        ident_sb = pp.tile([128, 128], F32, tag="ident")
        bqk_sb = pp.tile([128, 8], F32, tag="bqk")
        vbias_sb = pp.tile([128, HPC], F32, tag="vbias")
        bn_s = pp.tile([128, HPC], F32, tag="bn_s")
        bn_b = pp.tile([128, HPC], F32, tag="bn_b")
        bnc_s = pp.tile([128, HPC], F32, tag="bnc_s")
        bnc_b = pp.tile([128, HPC], F32, tag="bnc_b")
        bnn1_sb = pp.tile([128, 8], F32, tag="bnn1")
        wspe_sb = pp.tile([128, HPC * 128], F32R, tag="wspe")
        ident_bf = pp.tile([128, 128], BF16, tag="ident_bf")

        nc.sync.dma_start(ones_sb[:], ones_d[:].bitcast(F32R))
        nc.vector.tensor_copy(ones_bf[:], ones_sb[:, 0:1].bitcast(F32))
        nc.sync.dma_start(ones_row[:], ones_sb[:, 0:1])
        nc.sync.dma_start(ident_sb[:], ident_d[:])
        nc.sync.dma_start(bqk_sb[:], bqk_d[:])
        nc.sync.dma_start(vbias_sb[:], vbias_d[:])
        nc.sync.dma_start(bn_s[:], bn_s_d[:])
        nc.sync.dma_start(bn_b[:], bn_b_d[:])
        nc.sync.dma_start(bnc_s[:], bnc_s_d[:])
        nc.sync.dma_start(bnc_b[:], bnc_b_d[:])
        nc.sync.dma_start(bnn1_sb[:], bnn1_d[:])
        nc.sync.dma_start(wspe_sb[:], wspe_d[:].bitcast(F32R))
        nc.vector.tensor_copy(ident_bf[:], ident_sb[:])

        # spe_bc tiles live from phase C1 until the C2 fold
        with tc.tile_pool(name="spb", bufs=4) as spb_pool, \
             tc.tile_pool(name="xa", bufs=1) as xa_pool:
          # resident fp8 inputs for qkv (DMA-queued after the first conv head's
          # tiles so phase C1 starts promptly)
          x8_sb = xa_pool.tile([64, 8 * 2 * N], FP8, tag="x8")
          wq8_sb = xa_pool.tile([64, 8 * EB * 2 * 128], FP8, tag="wq8")
          wv8_sb = xa_pool.tile([64, 8 * 2 * 16], FP8, tag="wv8")
          x8v = x8_sb[:].rearrange("p (t e n) -> p t e n", t=8, e=2)
          wq8v = wq8_sb[:].rearrange("p (t b e m) -> p t b e m", t=8, b=EB, e=2)
          wv8v = wv8_sb[:].rearrange("p (t e m) -> p t e m", t=8, e=2)

          spe_bcs = []

          # ---- phase C1: conv + spe; vpT := cbr^T ----
          with tc.tile_pool(name="cvin", bufs=2) as cvin_pool, \
               tc.tile_pool(name="cvw", bufs=2) as cvw_pool, \
               tc.tile_pool(name="cbr", bufs=2) as cbr_pool, \
               tc.tile_pool(name="scr", bufs=2) as scr_pool, \
               tc.tile_pool(name="pcol", bufs=2) as pcol_pool, \
               tc.tile_pool(name="cps", bufs=2, space="PSUM") as cps, \
               tc.tile_pool(name="tps", bufs=2, space="PSUM") as tps, \
               tc.tile_pool(name="sps", bufs=1, space="PSUM") as sps:
            for h in range(HPC):
                xp8 = cvin_pool.tile([64, 2 * NPAD], FP8, tag="xp")
                nc.sync.dma_start(
                    xp8[:], xpad8_d[:, h * 2 * NPAD:(h + 1) * 2 * NPAD])
                wcv8 = cvw_pool.tile([64, 9 * 2 * 128], FP8, tag="wcv")
                nc.sync.dma_start(
                    wcv8[:], wcv8_d[:, h * 9 * 2 * 128:(h + 1) * 9 * 2 * 128])
                xsp = scr_pool.tile([128, N], BF16, tag="xsp")
                nc.sync.dma_start(xsp[:], xspe_d[:, h * N:(h + 1) * N])
                if h == 1:
                    # bulk A-phase inputs queue behind the first two conv heads
                    for qtr in range(4):
                        nc.sync.dma_start(
                            x8_sb[:, qtr * 2 * 2 * N:(qtr + 1) * 2 * 2 * N],
                            x8_d[:, qtr * 2 * 2 * N:(qtr + 1) * 2 * 2 * N])
                elif h == 2:
                    nc.sync.dma_start(wq8_sb[:], wq8_d[:])
                    nc.sync.dma_start(wv8_sb[:], wv8_d[:])
                xp8v = xp8[:].rearrange("p (e i) -> p e i", e=2)
                wcv8v = wcv8[:].rearrange("p (o e m) -> p o e m", o=9, e=2)

                # spe branch: gelu(bn(x)) with running row-sum -> pooled
                scr = scr_pool.tile([128, N], BF16, tag="scr")
                pcol = pcol_pool.tile([128, 1], F32, tag="pcol")
                nc.scalar.activation(
                    scr[:].rearrange("p (r c) -> p r c", c=PS),
                    xsp[:].rearrange("p (r c) -> p r c", c=PS), AF.Gelu,
                    bias=bn_b[:, h:h + 1], scale=bn_s[:, h:h + 1],
                    accum_out=pcol[:],
                )
                pcol_r = pcol_pool.tile([128, 1], F32R, tag="pcolr")
                nc.vector.tensor_copy(pcol_r[:], pcol[:])
                ps_spe = sps.tile([1, 128], F32, tag="spe")
                nc.tensor.matmul(ps_spe[:], pcol_r[:],
                                 wspe_sb[:, h * 128:(h + 1) * 128], start=True, stop=True)
                nc.vector.tensor_copy(spe_row[:, h * 128:(h + 1) * 128], ps_spe[:])
                # broadcast spe over partitions: spe_bc[p, c] = spe[c]
                ps_bc = sps.tile([128, 128], F32, tag="bc")
                nc.tensor.matmul(ps_bc[:], ones_row[:],
                                 spe_row[:, h * 128:(h + 1) * 128], start=True, stop=True)
                spe_bc = spb_pool.tile([128, 128], F32, tag="spb")
                nc.vector.tensor_copy(spe_bc[:], ps_bc[:])
                spe_bcs.append(spe_bc)

                # conv branch: 9 shifted fp8-DoubleRow matmuls, bn+gelu
                cbr = cbr_pool.tile([128, N], BF16, tag="cbr")
                for rc in range(6):  # 8 output rows per chunk
                    r0 = rc * 8
                    pc = cps.tile([128, 8 * PS], F32, tag="cv")
                    for oi, (dy, dx) in enumerate(
                            [(a, b) for a in (-1, 0, 1) for b in (-1, 0, 1)]):
                        base = (r0 + 1 + dy) * PP + 1 + dx
                        rhs = xp8v[:, :, base:base + 8 * PP].rearrange(
                            "p e (r c) -> p e r c", c=PP)[:, :, :, 0:PS]
                        nc.tensor.matmul(pc[:].rearrange("p (r c) -> p r c", c=PS),
                                         wcv8v[:, oi], rhs,
                                         start=(oi == 0), stop=(oi == 8),
                                         perf_mode=DRM)
                    nc.scalar.activation(cbr[:, r0 * PS:(r0 + 8) * PS], pc[:],
                                         AF.Gelu, bias=bnc_b[:, h:h + 1],
                                         scale=bnc_s[:, h:h + 1])
                for jb in range(NJ):
                    pt = tps.tile([128, 128], BF16, tag="tp")
                    nc.tensor.transpose(pt[:], cbr[:, jb * 128:(jb + 1) * 128], ident_bf[:])
                    nc.vector.tensor_copy(
                        vpT_sb[:, (h * NJ + jb) * 128:(h * NJ + jb + 1) * 128], pt[:])

          # ---- phase A: qkv projection (+ folded vsum rows), fp8 DoubleRow ----
          with tc.tile_pool(name="vr", bufs=1) as vr_pool, \
               tc.tile_pool(name="qtmp", bufs=2) as qtmp_pool, \
               tc.tile_pool(name="qps", bufs=4, space="PSUM") as qps, \
               tc.tile_pool(name="vps", bufs=2, space="PSUM") as vps:
            # vsum first: phase B + the C2 fold then overlap the eb loop
            vrow4 = vr_pool.tile([4, N], F32)
            for (n0, nw) in CHUNKS:
                pv16 = vps.tile([16, 512], F32, tag="v16")
                for dt in range(8):
                    nc.tensor.matmul(pv16[:, 0:nw], wv8v[:, dt],
                                     x8v[:, dt, :, n0:n0 + nw],
                                     start=(dt == 0), stop=(dt == 7), perf_mode=DRM)
                nc.vector.tensor_copy(vrow4[:, n0:n0 + nw], pv16[0:4, 0:nw])

            # ---- phase B: vsum -> per-head columns via DRAM reshape ----
            with tc.tile_pool(name="vdr", bufs=1, space="DRAM") as vdr_pool:
                vdr = vdr_pool.tile([4, N], F32)
                nc.sync.dma_start(vdr[:], vrow4[:])
                for h in range(HPC):
                    nc.sync.dma_start(
                        vcol_all[:, h * NJ:(h + 1) * NJ],
                        vdr[h:h + 1, :].rearrange("o (j p) -> (o p) j", p=128))
                    nc.vector.tensor_scalar_add(
                        vcol_all[:, h * NJ:(h + 1) * NJ],
                        vcol_all[:, h * NJ:(h + 1) * NJ], vbias_sb[:, h:h + 1])

            # ---- phase C2: fold v_spe into V' (in place) ----
            for h in range(HPC):
                for jb in range(NJ):
                    sl = vpT_sb[:, (h * NJ + jb) * 128:(h * NJ + jb + 1) * 128]
                    nc.vector.scalar_tensor_tensor(
                        sl, spe_bcs[h][:], vcol_all[:, h * NJ + jb:h * NJ + jb + 1],
                        sl, MULT, ADD)

            for eb in range(EB):
                qt = qtmp_pool.tile([128, N], FP8, tag="qt")
                for (n0, nw) in CHUNKS:
                    pq = qps.tile([128, 512], F32, tag="q")
                    for dt in range(8):
                        nc.tensor.matmul(
                            pq[:, 0:nw], wq8v[:, dt, eb],
                            x8v[:, dt, :, n0:n0 + nw],
                            start=(dt == 0), stop=(dt == 7), perf_mode=DRM)
                    nc.scalar.activation(
                        qt[:, n0:n0 + nw], pq[:, 0:nw], AF.Identity,
                        bias=bqk_sb[:, eb:eb + 1])
                # partition fold 128 -> [64, 2]: channel d = 64e + p
                for e in range(2):
                    nc.sync.dma_start(
                        qk8_sb[:, (eb * 2 + e) * N:(eb * 2 + e + 1) * N],
                        qt[64 * e:64 * e + 64, :])

        # ---------- phases D+E (overlapped: nn1 runs chunk-major) ----------
        for h in range(HPC):
            nc.sync.dma_start(wnn_sb[:, h * D:(h + 1) * D],
                              wnn1_d[:, h * D:(h + 1) * D])
        qk8v = qk8_sb[:].rearrange("p (b e n) -> p b e n", b=EB, e=2)
        with tc.tile_pool(name="pt", bufs=3) as pt_pool, \
             tc.tile_pool(name="str", bufs=2) as s_pool, \
             tc.tile_pool(name="sums", bufs=2) as sum_pool, \
             tc.tile_pool(name="fin", bufs=3) as fin_pool, \
             tc.tile_pool(name="dram", bufs=1, space="DRAM") as dram, \
             tc.tile_pool(name="dps", bufs=2, space="PSUM") as dps, \
             tc.tile_pool(name="ops", bufs=2, space="PSUM") as ops, \
             tc.tile_pool(name="fps", bufs=2, space="PSUM") as fps:
            if use_cc:
                part = dram.tile([D, N], F32)
                rs0 = dram.tile([CH // 2, N], F32)
                rs1 = dram.tile([CH // 2, N], F32)
            groups = [[0, 1], [2, 3], [4, 5], [6, 7]]

            # ---- phase D: attention (chunk-major), software-pipelined with
            # phase E (nn1): the denominator-finish of head h and a slice of
            # the previous chunk's nn1 are emitted after the NEXT head's dots
            # so the PE sequencer never stalls the exp stream.
            def emit_denom_finish(st):
                (h, i0, iw, S, po) = st
                pm = fps.tile([128, 512], F32, tag="f")
                nc.tensor.matmul(pm[0:1, 0:iw], ones_bf[:], S[:, 0:iw],
                                 start=True, stop=True)
                rsb = sum_pool.tile([1, 512], F32R, tag="r")
                with nc.allow_low_precision(reason="f32r keeps full fp32 range"):
                    nc.vector.reciprocal(rsb[:, 0:iw], pm[0:1, 0:iw])
                rbs = sum_pool.tile([128, 512], F32, tag="rbs")
                nc.gpsimd.partition_broadcast(
                    rbs[:, 0:iw], rsb[:, 0:iw].bitcast(F32))
                nc.vector.tensor_tensor(
                    outT_sb[:, h * N + i0:h * N + i0 + iw],
                    po[:, 0:iw], rbs[:, 0:iw], MULT)

            def emit_nn1(ci, ebos):
                n0, nw = CHUNKS[ci]
                for ebo in ebos:
                    pf = fps.tile([128, 512], F32, tag="f")
                    for h in range(HPC):
                        nc.tensor.matmul(
                            pf[:, 0:nw],
                            wnn_sb[:, h * D + ebo * 128:h * D + (ebo + 1) * 128],
                            outT_sb[:, h * N + n0:h * N + n0 + nw],
                            start=(h == 0), stop=(h == HPC - 1))
                    fin = fin_pool.tile([128, 512], F32, tag="fin")
                    nc.vector.tensor_scalar_add(fin[:, 0:nw], pf[:, 0:nw],
                                                bnn1_sb[:, ebo:ebo + 1])
                    dst = part if use_cc else out_d
                    nc.sync.dma_start(dst[ebo * 128:(ebo + 1) * 128, n0:n0 + nw],
                                      fin[:, 0:nw])

            if dump:
                nc.sync.dma_start(dbg_qk_d, qk8_sb[:])
                nc.sync.dma_start(dbg_vpT_d, vpT_sb[:])
                nc.sync.dma_start(dbg_vcol_d, vcol_all[:])
                nc.sync.dma_start(dbg_spe_d, spe_row[:].bitcast(F32))
            pending = None
            for ci, (i0, iw) in enumerate(CHUNKS):
                for h in range(HPC):
                    pt_all = pt_pool.tile([128, NJ * 512], BF16, tag="pt")
                    for jp in range(NJ // 2):
                        pd = dps.tile([128, 1024], F32, tag="d")
                        for e in range(2):
                            jb = 2 * jp + e
                            nc.tensor.matmul(
                                pd[:, e * 512:e * 512 + iw],
                                qk8v[:, HPC + h, :, jb * 128:(jb + 1) * 128],
                                qk8v[:, h, :, i0:i0 + iw],
                                start=True, stop=True, perf_mode=DRM)
                        if iw == 512:
                            nc.scalar.activation(
                                pt_all[:, jp * 1024:(jp + 1) * 1024],
                                pd[:], AF.Exp, scale=SCALE)
                        else:
                            for e in range(2):
                                jb = 2 * jp + e
                                nc.scalar.activation(
                                    pt_all[:, jb * 512:jb * 512 + iw],
                                    pd[:, e * 512:e * 512 + iw], AF.Exp,
                                    scale=SCALE)
                    po = ops.tile([128, 512], F32, tag="o")
                    for jb in range(NJ):
                        nc.tensor.matmul(
                            po[:, 0:iw],
                            vpT_sb[:, (h * NJ + jb) * 128:(h * NJ + jb + 1) * 128],
                            pt_all[:, jb * 512:jb * 512 + iw],
                            start=(jb == 0), stop=(jb == NJ - 1))
                    # finish the PREVIOUS head's softmax (its DVE tree is done
                    # by now, so the ones-matmul doesn't stall the PE).  The
                    # nn1 slice for the previous chunk must come after it: at
                    # (ci, h=0) the pending finish is the previous chunk's h3
                    # outT write, which nn1 reads.
                    if pending is not None:
                        emit_denom_finish(pending)
                    if ci > 0:
                        emit_nn1(ci - 1, [2 * h, 2 * h + 1])
                    # denominator: 18-block sum as a DVE tree into S
                    S = s_pool.tile([128, 9 * 512], BF16, tag="S")
                    if iw == 512:
                        nc.vector.tensor_tensor(
                            S[:], pt_all[:, 0:9 * 512],
                            pt_all[:, 9 * 512:18 * 512], ADD)
                        nc.vector.tensor_tensor(
                            S[:, 0:4 * 512], S[:, 0:4 * 512],
                            S[:, 4 * 512:8 * 512], ADD)
                        nc.vector.tensor_tensor(
                            S[:, 0:2 * 512], S[:, 0:2 * 512],
                            S[:, 2 * 512:4 * 512], ADD)
                        nc.vector.tensor_tensor(
                            S[:, 0:512], S[:, 0:512], S[:, 512:1024], ADD)
                        nc.vector.tensor_tensor(
                            S[:, 0:512], S[:, 0:512], S[:, 8 * 512:9 * 512], ADD)
                    else:
                        ptv = pt_all[:].rearrange("p (n w) -> p n w", w=512)
                        Sv = S[:].rearrange("p (n w) -> p n w", w=512)
                        nc.vector.tensor_tensor(
                            Sv[:, 0:9, 0:iw], ptv[:, 0:9, 0:iw],
                            ptv[:, 9:18, 0:iw], ADD)
                        nc.vector.tensor_tensor(
                            Sv[:, 0:4, 0:iw], Sv[:, 0:4, 0:iw],
                            Sv[:, 4:8, 0:iw], ADD)
                        nc.vector.tensor_tensor(
                            Sv[:, 0:2, 0:iw], Sv[:, 0:2, 0:iw],
                            Sv[:, 2:4, 0:iw], ADD)
                        nc.vector.tensor_tensor(
                            Sv[:, 0:1, 0:iw], Sv[:, 0:1, 0:iw],
                            Sv[:, 1:2, 0:iw], ADD)
                        nc.vector.tensor_tensor(
                            Sv[:, 0:1, 0:iw], Sv[:, 0:1, 0:iw],
                            Sv[:, 8:9, 0:iw], ADD)
                    pending = (h, i0, iw, S, po)
            emit_denom_finish(pending)
            if dump:
                nc.sync.dma_start(dbg_outT_d, outT_sb[:])
            emit_nn1(4, [0, 1, 2, 3])
            if use_cc:
                nc.gpsimd.collective_compute(
                    "ReduceScatter", ADD, replica_groups=groups,
                    ins=[part[0:CH, :].opt()], outs=[rs0[:].opt()])
                nc.sync.dma_start(out_d[0:CH // 2, :], rs0[:])
            emit_nn1(4, [4, 5, 6, 7])
            if use_cc:
                nc.gpsimd.collective_compute(
                    "ReduceScatter", ADD, replica_groups=groups,
                    ins=[part[CH:D, :].opt()], outs=[rs1[:].opt()])
                nc.sync.dma_start(out_d[CH // 2:CH, :], rs1[:])

    nc.compile()
    return nc


def _host_inputs(core, inp):
    b, half = core // 2, core % 2
    h0 = half * HPC
    FP8NP = mybir.dt.np(FP8)
    x = np.asarray(inp["x"][b], dtype=np.float32)            # (D, N)
    Wqkv = np.asarray(inp["Wqkv"], dtype=np.float32)
    bqkv = np.asarray(inp["bqkv"], dtype=np.float32)
    Wspe = np.asarray(inp["Wspe"], dtype=np.float32)[:, :, 0, 0]   # (D, H)
    Wlocal = np.asarray(inp["Wlocal"], dtype=np.float32)     # (D, 8, 3, 3)
    Wnn1 = np.asarray(inp["Wnn1"], dtype=np.float32)
    bnn1 = np.asarray(inp["bnn1"], dtype=np.float32)

    chs = slice(h0 * HD, (h0 + HPC) * HD)                    # this core's 512 channels

    # x in DoubleRow layout: x8[p, t, e, n] = x[128t + 64e + p, n]
    x8 = x.reshape(8, 2, 64, N).transpose(2, 0, 1, 3).reshape(64, 8 * 2 * N)
    x8 = x8.astype(FP8NP)

    # image layout: reinterpret x^T flat as (D, 48, 48); pad to 50x50
    ximg = np.ascontiguousarray(x.T).reshape(D, N)[chs]      # (512, 2304)
    pad = np.zeros((CH, NPAD), np.float32)
    pad3 = pad[:, :PP * PP].reshape(CH, PP, PP)
    pad3[:, 1:PS + 1, 1:PS + 1] = ximg.reshape(CH, PS, PS)
    # xpad8[p, h, e, i] = pad[h*128 + 64e + p, i]
    xpad8 = pad.reshape(HPC, 2, 64, NPAD).transpose(2, 0, 1, 3).reshape(
        64, HPC * 2 * NPAD).astype(FP8NP)
    # bf16 image for the spe gelu+pool branch: [128, (h, n)], channel-partition
    xspe = np.ascontiguousarray(
        ximg.reshape(HPC, 128, N).transpose(1, 0, 2).reshape(128, HPC * N)
    ).astype(ml_dtypes.bfloat16)

    # qkv weights: e-blocks = [q heads, k heads], DR lhsT layout
    rows = np.concatenate(
        [np.arange(h0 * HD, (h0 + HPC) * HD) + s * D for s in range(2)])
    wqkvT = Wqkv[rows, :].T                                   # (1024 d_in, 1024 ch)
    # wq8[p, t, eb, e, m] = wqkvT[128t + 64e + p, 128 eb + m]
    wq8 = wqkvT.reshape(8, 2, 64, EB, 128).transpose(2, 0, 3, 1, 4).reshape(
        64, 8 * EB * 2 * 128).astype(FP8NP)
    bqk = bqkv[rows].reshape(8, 128).T.copy()                 # q,k biases (128, 8)
    vrows = np.arange(h0 * HD, (h0 + HPC) * HD) + 2 * D
    wv = Wqkv[vrows, :].reshape(HPC, 128, D).sum(axis=1)      # (HPC, 1024)
    wvT = wv.T                                                # (1024, HPC)
    wv16 = np.zeros((D, 16), np.float32)
    wv16[:, 0:HPC] = wvT
    wv8 = wv16.reshape(8, 2, 64, 16).transpose(2, 0, 1, 3).reshape(
        64, 8 * 2 * 16).astype(FP8NP)
    vb = bqkv[vrows].reshape(HPC, 128).sum(axis=1)            # summed v bias per head
    vbias = np.repeat(vb[None, :], 128, axis=0).astype(np.float32)

    # dense per-head conv weights, DR lhsT[cin-pair, cout] per (head, offset)
    wconv = np.zeros((HPC, 9, 128, 128), np.float32)          # [h, oi, cin, cout]
    for h in range(HPC):
        for co in range(128):
            g = co // 8
            cg = np.arange(g * 8, g * 8 + 8)
            for oi, (dy, dx) in enumerate(
                    [(a, c) for a in range(3) for c in range(3)]):
                wconv[h, oi, cg, co] = Wlocal[(h0 + h) * HD + co, :, dy, dx]
    # wcv8[p, h, oi, e, m] = wconv[h, oi, 64e + p, m]
    wcv8 = wconv.reshape(HPC, 9, 2, 64, 128).transpose(3, 0, 1, 2, 4).reshape(
        64, HPC * 9 * 2 * 128).astype(FP8NP)

    # spe block-diag matrix (folds in 1/N pooling mean and attention scale)
    wspe = np.zeros((HPC, 128, 128), np.float32)              # [h, c_in, idx]
    for h in range(HPC):
        for gg in range(16):
            g = (h0 + h) * 16 + gg
            blk = Wspe[g * 8:(g + 1) * 8, :8]                 # [o, i]
            wspe[h, gg * 8:gg * 8 + 8, gg * 8:gg * 8 + 8] = blk.T  # [i, o]
    wspe = (wspe * (SCALE / N)).transpose(1, 0, 2).reshape(128, HPC * 128)

    def fold_bn(g, bta, mu, var):
        s = np.asarray(g, np.float64) / np.sqrt(np.asarray(var, np.float64) + EPS)
        return (s.astype(np.float32),
                (np.asarray(bta, np.float64) - np.asarray(mu, np.float64) * s)
                .astype(np.float32))

    bn_s, bn_b = fold_bn(inp["bn_gamma"], inp["bn_beta"], inp["bn_mean"], inp["bn_var"])
    bnc_s, bnc_b = fold_bn(inp["bnc_gamma"], inp["bnc_beta"], inp["bnc_mean"],
                           inp["bnc_var"])
    shp = lambda a: np.ascontiguousarray(a[chs].reshape(HPC, 128).T)

    wnn1T = Wnn1[:, chs].T                                    # (512, 1024)
    wnn1 = wnn1T.reshape(HPC, 128, D).transpose(1, 0, 2).reshape(128, HPC * D)
    wnn1 = wnn1.astype(ml_dtypes.bfloat16)
    bnn1h = np.ascontiguousarray((0.5 * bnn1).reshape(8, 128).T)

    ones = np.ones((128, 2), np.float32)
    ident = np.eye(128, dtype=np.float32)
    return {
        "x8": np.ascontiguousarray(x8), "xspe": xspe, "xpad8": xpad8,
        "wq8": np.ascontiguousarray(wq8), "bqk": np.ascontiguousarray(bqk),
        "wv8": np.ascontiguousarray(wv8),
        "vbias": vbias, "wcv8": np.ascontiguousarray(wcv8),
        "wspe": np.ascontiguousarray(wspe),
        "bn_s": shp(bn_s), "bn_b": shp(bn_b),
        "bnc_s": shp(bnc_s), "bnc_b": shp(bnc_b),
        "wnn1": np.ascontiguousarray(wnn1), "bnn1h": bnn1h,
        "onesc": ones, "ident": ident,
    }


_NC = None


def kernel(**inputs):
    global _NC
    if _NC is None:
        _NC = _build()
    in_maps = [_host_inputs(c, inputs) for c in range(8)]
    res = run_bass_kernel_spmd(_NC, in_maps, core_ids=list(range(8)))
    out = np.empty((B, N, D), np.float32)
    for b in range(B):
        if USE_COLLECTIVE:
            ev, od = res.results[2 * b]["out"], res.results[2 * b + 1]["out"]
            t = np.empty((D, N), np.float32)
            t[0:256] = ev[0:256]
            t[256:512] = od[0:256]
            t[512:768] = ev[256:512]
            t[768:1024] = od[256:512]
        else:
            t = res.results[2 * b]["out"] + res.results[2 * b + 1]["out"]
        out[b] = t.T
    return out


def run_timed(**inputs):
    """Re-run with NTFF tracing to get HW exec time (best effort)."""
    global _NC
    if _NC is None:
        _NC = _build()
    in_maps = [_host_inputs(c, inputs) for c in range(8)]
    try:
        return run_bass_kernel_spmd(_NC, in_maps, core_ids=list(range(8)), trace=True)
    except Exception as e:  # tracing unsupported under some axon terminals
        print(f"trace run failed: {e}")
        return None
